# revision 1
# baseline (speedup 1.0000x reference)
"""DynamicsNet Trainium2 kernel: 4 zero-state LSTM cells, data-parallel on 8 cores.

Reference math per row x[16]:
    h1 = relu(lstm1(x));  h2 = selu(lstm2(h1));  m = tanh(lstmM(h2));
    d = tanh(lstmD(h2));  out = concat([m, d], axis=0)
(zero-state LSTM cell: h = sigmoid(o) * tanh(sigmoid(i) * tanh(g)), f unused)

Layout: per core, 8 chunk-streams; stream j owns partitions 16j..16j+12 with a
4-partition gap. Gates are produced gate-bank-major (I/G/O in separate PSUM
banks, same partitions) so every elementwise op is partition-aligned [128, F].
Each gate bank is ONE full-array matmul: the stationary is an 8-way
block-diagonal [128, 128] with per-chunk W.T blocks, a bias row on the
constant gap partition, and gap-column constants engineered so downstream
h-tile gap lanes hold an exact known constant (the next cell's bias 'one').

Precision: fp16 hi/lo 3-term split (W16*x16 + Wlo*x16 + W16*xlo) recovers
fp32-grade matmul accuracy at 16-bit PE speed. Sigmoid goes through tanh:
sigma(z) = (1+tanh(z/2))/2 with the 1/2 folded into weights, so only
Tanh+Exp ACT tables are used. selu is refactored to
(lambda/2 W)*(max(h2x,0) + 2a*exp(min(h2x,0)/2)) with the -a*lambda shift
folded into the m/d biases.
"""

from contextlib import ExitStack

import numpy as np

LAMBDA = 1.0507009873554805
ALPHA = 1.6732632423543772
AL = ALPHA * LAMBDA

B, IN, H = 1048576, 16, 12
NCORES = 8
R = B // NCORES          # rows per core
NCHUNK = 8               # chunk streams per core
CLEN = R // NCHUNK       # 16384 rows per stream
F = 512                  # free-dim tile
NIT = CLEN // F          # 32 iterations
GAP_A = 2.0              # gap bias for I and O banks
V2 = 1.25                # engineered H2 gap constant (fp16-exact)

_CACHED = {}


def _solve_gap_g(target):
    """Gap bias for the G bank so gap-lane h2x equals `target`."""
    t_a = np.tanh(GAP_A)
    tc = target / (1.0 + t_a)
    c2 = 2.0 * np.arctanh(tc)
    tg = c2 / (1.0 + t_a)
    assert abs(tg) < 1.0
    return float(np.arctanh(tg))


def _split16(a):
    hi = a.astype(np.float16)
    lo = (a - hi.astype(np.float64)).astype(np.float16)
    return hi, lo


def _stats_cell1(wb):
    """3 stationaries [128,128] fp16 for the x-input cell (no bias rows).
    Terms: W16*x_hi, Wlo*x_hi, W16*x_lo."""
    w16, wlo = _split16(wb)
    outs = []
    for w in (w16, wlo, w16):
        m = np.zeros((128, 128), np.float16)
        for j in range(NCHUNK):
            m[16 * j:16 * j + 16, 16 * j:16 * j + 12] = w.T
        outs.append(m)
    return outs


def _stats_hcell(wb, bb, gapval, vgap):
    """3 stationaries [128,128] fp16 for h-input cells.
    Terms: W16*H_hi (+bias rows), Wlo*H_hi (unused in 2-term mode),
    W16*H_lo. The bias rides H_hi gap lanes 12 AND 13 (both engineered to
    the constant `vgap`), split hi/lo across two stationary rows for
    fp32-grade bias accuracy. `gapval` seeds this cell's own gap columns."""
    w16, wlo = _split16(wb)
    bv = np.asarray(bb, np.float64) / vgap
    b16, blo = _split16(bv)
    outs = []
    for term, w in enumerate((w16, wlo, w16)):
        m = np.zeros((128, 128), np.float16)
        for j in range(NCHUNK):
            m[16 * j:16 * j + 12, 16 * j:16 * j + 12] = w.T
            if term == 0:
                m[16 * j + 12, 16 * j:16 * j + 12] = b16
                m[16 * j + 13, 16 * j:16 * j + 12] = blo
                m[16 * j + 12, 16 * j + 12:16 * j + 16] = gapval
        outs.append(m)
    return outs


def _prepare_consts(W_ih1, b_ih1, b_hh1, W_ih2, b_ih2, b_hh2,
                    W_ihm, b_ihm, b_hhm, W_ihd, b_ihd, b_hhd):
    i_s, g_s, o_s = slice(0, 12), slice(24, 36), slice(36, 48)
    g1gap = _solve_gap_g(1.0)    # H1 gap -> 1.0
    g2gap = _solve_gap_g(V2)     # H2c gap -> V2 (positive branch: h2x == H2c)

    b1 = (b_ih1 + b_hh1).astype(np.float64)
    b2 = (b_ih2 + b_hh2).astype(np.float64)
    # h2 = (lambda/2)*H2c + (al - al) ... with H2c centered the al-shift cancels:
    # W*h2 = (lambda/2)*W*H2c + al*sum(W) - al*sum(W) = (lambda/2)*W*H2c
    bm = (b_ihm + b_hhm).astype(np.float64)
    bd = (b_ihd + b_hhd).astype(np.float64)

    W1 = W_ih1.astype(np.float64)
    W2 = W_ih2.astype(np.float64)
    Wm = W_ihm.astype(np.float64)
    Wd = W_ihd.astype(np.float64)
    L2 = LAMBDA / 2.0

    # stat[(cell, bank, term)]; bank 0=I(sig), 1=G, 2=O(sig)
    stats = {}
    for bank, (gsl, sc) in enumerate(((i_s, 0.5), (g_s, 1.0), (o_s, 0.5))):
        gv1 = {0: GAP_A, 1: g1gap, 2: GAP_A}[bank]
        gv2 = {0: GAP_A, 1: g2gap, 2: GAP_A}[bank]
        stats[(0, bank)] = _stats_cell1(W1[gsl] * sc)
        # cell2 input H1 = 2*relu(h1) -> extra 1/2
        stats[(1, bank)] = _stats_hcell(W2[gsl] * (sc * 0.5), b2[gsl] * sc,
                                        gv2, 1.0)
        # m/d input H2'' -> scale lambda/2; gap lanes don't matter (0)
        stats[(2, bank)] = _stats_hcell(Wm[gsl] * (sc * L2), bm[gsl] * sc,
                                        0.0, V2)
        stats[(3, bank)] = _stats_hcell(Wd[gsl] * (sc * L2), bd[gsl] * sc,
                                        0.0, V2)

    w_np = np.zeros((128, 36 * 128), np.float16)
    for cell in range(4):
        for bank in range(3):
            for term in range(3):
                s = cell * 9 + bank * 3 + term
                w_np[:, 128 * s:128 * s + 128] = stats[(cell, bank)][term]

    # cell1 per-partition ACT bias: [128, 3] cols = I, G, O banks
    b_np = np.zeros((128, 3), np.float32)
    for j in range(NCHUNK):
        sl = slice(16 * j, 16 * j + 12)
        b_np[sl, 0] = b1[i_s] * 0.5
        b_np[sl, 1] = b1[g_s]
        b_np[sl, 2] = b1[o_s] * 0.5
        gp = slice(16 * j + 12, 16 * j + 16)
        b_np[gp, 0] = GAP_A
        b_np[gp, 1] = g1gap
        b_np[gp, 2] = GAP_A
    return w_np, b_np


def _build_bass():
    import concourse.bass as bass
    import concourse.mybir as mybir
    import concourse.tile as tile

    fp32 = mybir.dt.float32
    fp16 = mybir.dt.float16
    Tanh = mybir.ActivationFunctionType.Tanh
    Exp = mybir.ActivationFunctionType.Exp
    ADD = mybir.AluOpType.add
    MULT = mybir.AluOpType.mult
    MAX = mybir.AluOpType.max
    SUB = mybir.AluOpType.subtract
    TWOA = float(2.0 * ALPHA)

    nc = bass.Bass(name="dynet")
    xh_dev = nc.dram_tensor("xh_dev", [128, CLEN], fp16, kind="ExternalInput")
    xl_dev = nc.dram_tensor("xl_dev", [128, CLEN], fp16, kind="ExternalInput")
    w_dram = nc.dram_tensor("w_dram", [128, 36 * 128], fp16, kind="ExternalInput")
    b_dram = nc.dram_tensor("b_dram", [128, 3], fp32, kind="ExternalInput")
    m_dev = nc.dram_tensor("m_dev", [128, CLEN], fp32, kind="ExternalOutput")
    d_dev = nc.dram_tensor("d_dev", [128, CLEN], fp32, kind="ExternalOutput")

    with tile.TileContext(nc) as tc, ExitStack() as ctx:
        const_p = ctx.enter_context(tc.tile_pool(name="const", bufs=1))
        xp = ctx.enter_context(tc.tile_pool(name="x", bufs=3))
        Tp = ctx.enter_context(tc.tile_pool(name="T", bufs=3))
        smallp = ctx.enter_context(tc.tile_pool(name="small", bufs=4))
        mdp = ctx.enter_context(tc.tile_pool(name="md", bufs=3))
        hp = ctx.enter_context(tc.tile_pool(name="h", bufs=4))
        psp = ctx.enter_context(tc.tile_pool(name="ps", bufs=1, space="PSUM"))

        wsb = const_p.tile([128, 36 * 128], fp16)
        nc.sync.dma_start(wsb[:], w_dram[:])
        bsb = const_p.tile([128, 3], fp32)
        nc.sync.dma_start(bsb[:], b_dram[:])
        ebt = const_p.tile([128, 1], fp32)
        nc.vector.memset(ebt[:], float(np.log(2.0 * ALPHA)))

        # PSUM as a ring of 8 banks; each stage claims 3 consecutive (mod 8).
        # Rolling reuse keeps the PE ~2.7 stages ahead of the WAR horizon so
        # it never stalls long enough for the HAM clock gate to re-throttle.
        pp_all = psp.tile([128, 4096], fp32)

        def bank(s, b):
            w = ((3 * s + b) % 8) * 512
            return pp_all[:, w:w + 512]

        def mms(s, cell, rhs_hi, rhs_lo):
            """6 full-array MMs: 3 banks x 2 accumulating fp16 terms
            (W16*hi with exact split bias rows, W16*lo; the W-residual
            term is dropped: ~1e-5 output error, measured)."""
            for term, rhs in ((0, rhs_hi), (2, rhs_lo)):
                for b in range(3):
                    w = cell * 9 + b * 3 + term
                    nc.tensor.matmul(
                        bank(s, b), wsb[:, 128 * w:128 * w + 128], rhs[:, :],
                        start=(term == 0), stop=(term == 2))

        def gate_act(s, T, bias_ap=None):
            """tanh over the stage's 3 ring banks -> T[:, 0:1536]."""
            base = (3 * s) % 8
            if bias_ap is not None:
                for b in range(3):
                    nc.scalar.activation(T[:, 512 * b:512 * b + 512],
                                         bank(s, b), Tanh,
                                         bias=bias_ap[:, b:b + 1])
                return
            n1 = min(3, 8 - base)
            nc.scalar.activation(
                T[:, 0:512 * n1],
                pp_all[:, 512 * base:512 * (base + n1)], Tanh)
            if n1 < 3:
                nc.scalar.activation(
                    T[:, 512 * n1:1536], pp_all[:, 0:512 * (3 - n1)], Tanh)

        xh = {}
        xl = {}
        H1 = {}
        H2 = {}
        sctr = 0
        for k in range(NIT + 3):
            # prefetch x for iteration k
            if k < NIT:
                xh[k] = xp.tile([128, F], fp16, tag="xh", name=f"xh{k}")
                nc.sync.dma_start(xh[k][:], xh_dev[:, F * k:F * (k + 1)])
                xl[k] = xp.tile([128, F], fp16, tag="xl", name=f"xl{k}")
                nc.sync.dma_start(xl[k][:], xl_dev[:, F * k:F * (k + 1)])

            # fused tanh input/output: [0:1024]=hmd(k-3), [1024:2048]=c2both(k)
            fin = smallp.tile([128, 2048], fp32, tag="fin", name=f"fin{k}")
            fout = smallp.tile([128, 2048], fp32, tag="fout", name=f"fout{k}")
            hmd = fin[:, 0:1024]
            omd = fout[:, 0:1024]
            c2both = fin[:, 1024:2048]
            tcboth = fout[:, 1024:2048]
            # --- stage m/d for iteration k-3 ---
            if 3 <= k:
                it = k - 3
                sm, sd = sctr, sctr + 1
                sctr += 2
                mms(sm, 2, H2[it][0], H2[it][1])
                mms(sd, 3, H2[it][0], H2[it][1])
                Tmd = Tp.tile([128, 3072], fp32, tag="Tmd", bufs=2)
                Tm = Tmd[:, 0:1536]
                Td = Tmd[:, 1536:3072]
                c2md = mdp.tile([128, 1024], fp32, tag="c2md")
                tcmd = mdp.tile([128, 1024], fp32, tag="tcmd")
                if (3 * sm) % 8 <= 2:
                    # m and d banks are 6 contiguous banks: one tanh op
                    base = (3 * sm) % 8
                    nc.scalar.activation(
                        Tmd[:, 0:3072],
                        pp_all[:, 512 * base:512 * base + 3072], Tanh)
                else:
                    gate_act(sm, Tm)
                    gate_act(sd, Td)
                nc.vector.scalar_tensor_tensor(
                    c2md[:, 0:512], Tm[:, 0:512], 1.0, Tm[:, 512:1024],
                    op0=ADD, op1=MULT)
                nc.vector.scalar_tensor_tensor(
                    c2md[:, 512:1024], Td[:, 0:512], 1.0, Td[:, 512:1024],
                    op0=ADD, op1=MULT)
                nc.scalar.activation(tcmd[:, :], c2md[:, :], Tanh, scale=0.5)
                nc.vector.scalar_tensor_tensor(
                    hmd[:, 0:512], Tm[:, 1024:1536], 1.0, tcmd[:, 0:512],
                    op0=ADD, op1=MULT)
                nc.vector.scalar_tensor_tensor(
                    hmd[:, 512:1024], Td[:, 1024:1536], 1.0, tcmd[:, 512:1024],
                    op0=ADD, op1=MULT)
                del H2[it]
                md_out_it = it


            # --- stage cell2 for iteration k-1 (produce c2, defer tc) ---
            if 1 <= k <= NIT:
                it = k - 1
                s2 = sctr
                sctr += 1
                mms(s2, 1, H1[it][0], H1[it][1])
                T2 = Tp.tile([128, 1536], fp32, tag="T")
                gate_act(s2, T2)
                nc.vector.scalar_tensor_tensor(
                    c2both[:, 0:512], T2[:, 0:512], 1.0, T2[:, 512:1024],
                    op0=ADD, op1=MULT)

            # --- stage cell1 for iteration k ---
            if k < NIT:
                s1 = sctr
                sctr += 1
                mms(s1, 0, xh[k], xl[k])
                T1 = Tp.tile([128, 1536], fp32, tag="T")
                gate_act(s1, T1, bias_ap=bsb)
                nc.vector.scalar_tensor_tensor(
                    c2both[:, 512:1024], T1[:, 0:512], 1.0, T1[:, 512:1024],
                    op0=ADD, op1=MULT)

            # merged tanh(0.5*x) over [hmd(k-3) | c2_2(k-1) | c2_1(k)]
            lo = 0 if 3 <= k else 1024
            hi = 2048 if k < NIT else (1536 if k <= NIT else 1024)
            if k == 0:
                lo = 1536
            nc.scalar.activation(fout[:, lo:hi], fin[:, lo:hi], Tanh,
                                 scale=0.5)
            if 3 <= k:
                it = md_out_it
                nc.sync.dma_start(m_dev[:, F * it:F * (it + 1)], omd[:, 0:512])
                nc.sync.dma_start(d_dev[:, F * it:F * (it + 1)], omd[:, 512:1024])

            # --- cell2 epilogue: h2x -> H2'' hi/lo ---
            if 1 <= k <= NIT:
                it = k - 1
                h2x2 = smallp.tile([128, F], fp32, tag="h2x2",
                                   name=f"h2x2_{k}")
                nc.vector.scalar_tensor_tensor(
                    h2x2[:, :], T2[:, 1024:1536], 1.0, tcboth[:, 0:512],
                    op0=ADD, op1=MULT)
                m0t = smallp.tile([128, F], fp32, tag="m0")
                e2t = smallp.tile([128, F], fp32, tag="e2")
                tmp2 = smallp.tile([128, F], fp32, tag="tmp2")
                h2h = hp.tile([128, F], fp16, tag="H2h", name=f"H2h_{it}")
                h2l = hp.tile([128, F], fp16, tag="H2l", name=f"H2l_{it}")
                nc.vector.tensor_scalar_min(m0t[:, :], h2x2[:, :], 0.0)
                nc.scalar.activation(e2t[:, :], m0t[:, :], Exp,
                                     bias=ebt[:, :], scale=0.5)
                nc.vector.scalar_tensor_tensor(
                    tmp2[:, :], h2x2[:, :], 0.0, e2t[:, :], op0=MAX, op1=ADD)
                nc.vector.tensor_scalar_sub(h2h[:, :], tmp2[:, :], TWOA)
                nc.vector.scalar_tensor_tensor(
                    h2l[:, :], tmp2[:, :], TWOA, h2h[:, :], op0=SUB, op1=SUB)
                H2[it] = (h2h, h2l)
                del H1[it]

            # --- cell1 epilogue: h2x -> H1 hi/lo ---
            if k < NIT:
                h2x1 = smallp.tile([128, F], fp32, tag="h2x1",
                                   name=f"h2x1_{k}")
                nc.vector.scalar_tensor_tensor(
                    h2x1[:, :], T1[:, 1024:1536], 1.0, tcboth[:, 512:1024],
                    op0=ADD, op1=MULT)
                h1h = hp.tile([128, F], fp16, tag="H1h", name=f"H1h_{k}")
                h1l = hp.tile([128, F], fp16, tag="H1l", name=f"H1l_{k}")
                nc.vector.tensor_scalar_max(h1h[:, :], h2x1[:, :], 0.0)
                nc.vector.scalar_tensor_tensor(
                    h1l[:, :], h2x1[:, :], 0.0, h1h[:, :], op0=MAX, op1=SUB)
                H1[k] = (h1h, h1l)
                del xh[k], xl[k]

    _legalize_waits(nc)
    return nc


def _legalize_waits(nc):
    """Split multi-wait instructions into single-wait same-engine NoOps
    (the cayman ISA has one sync-wait slot per instruction)."""
    import concourse.mybir as mybir
    n = 0
    for func in nc.m.functions:
        for blk in func.blocks:
            out = []
            changed = False
            for inst in blk.instructions:
                si = inst.sync_info
                waits = list(si.on_wait) if si is not None and si.on_wait else []
                if len(waits) > 1:
                    changed = True
                    for w in waits[:-1]:
                        n += 1
                        nop = mybir.InstNoOp(name=f"legw-{n}", ins=[], outs=[])
                        nop.engine = inst.engine
                        nop.sync_info = mybir.SyncInfo(on_wait=[w], on_update=[])
                        out.append(nop)
                    inst.sync_info = mybir.SyncInfo(
                        on_wait=[waits[-1]],
                        on_update=list(si.on_update) if si.on_update else [])
                out.append(inst)
            if changed:
                blk.instructions = out
    return n


def _run(x, consts, trace=False):
    from concourse.bass_utils import run_bass_kernel_spmd

    if "nc" not in _CACHED:
        _CACHED["nc"] = _build_bass()
    nc = _CACHED["nc"]
    w_np, b_np = consts

    in_maps = []
    for c in range(NCORES):
        xs = x[c * R:(c + 1) * R].reshape(NCHUNK, CLEN, IN)
        x_devc = np.ascontiguousarray(
            xs.transpose(0, 2, 1)).reshape(128, CLEN).astype(np.float64)
        xhi = x_devc.astype(np.float16)
        xlo = (x_devc - xhi.astype(np.float64)).astype(np.float16)
        in_maps.append({"xh_dev": xhi, "xl_dev": xlo,
                        "w_dram": w_np, "b_dram": b_np})

    res = run_bass_kernel_spmd(nc, in_maps, core_ids=list(range(NCORES)),
                               trace=trace)

    out = np.empty((2 * B, H), np.float32)
    for c in range(NCORES):
        for name, base in (("m_dev", 0), ("d_dev", B)):
            dev = res.results[c][name]  # [128, CLEN]
            full = dev.reshape(NCHUNK, 16, CLEN)[:, :12, :]
            out[base + c * R: base + (c + 1) * R] = (
                full.transpose(0, 2, 1).reshape(R, H))
    return out, res


def kernel(x, W_ih1, b_ih1, b_hh1, W_ih2, b_ih2, b_hh2,
           W_ihm, b_ihm, b_hhm, W_ihd, b_ihd, b_hhd):
    x = np.asarray(x, np.float32)
    consts = _prepare_consts(
        np.asarray(W_ih1, np.float32), np.asarray(b_ih1, np.float32),
        np.asarray(b_hh1, np.float32), np.asarray(W_ih2, np.float32),
        np.asarray(b_ih2, np.float32), np.asarray(b_hh2, np.float32),
        np.asarray(W_ihm, np.float32), np.asarray(b_ihm, np.float32),
        np.asarray(b_hhm, np.float32), np.asarray(W_ihd, np.float32),
        np.asarray(b_ihd, np.float32), np.asarray(b_hhd, np.float32))
    out, _ = _run(x, consts, trace=False)
    return out



# revision 10
# speedup vs baseline: 1.3330x; 1.3330x over previous
"""DynamicsNet Trainium2 kernel: 4 zero-state LSTM cells, data-parallel on 8 cores.

Reference math per row x[16]:
    h1 = relu(lstm1(x));  h2 = selu(lstm2(h1));  m = tanh(lstmM(h2));
    d = tanh(lstmD(h2));  out = concat([m, d], axis=0)
(zero-state LSTM cell: h = sigmoid(o) * tanh(sigmoid(i) * tanh(g)), f unused)

Layout: per core, 8 chunk-streams; stream j owns partitions 16j..16j+12 with a
4-partition gap. Gates are produced gate-bank-major (I/G/O in separate PSUM
banks, same partitions) so every elementwise op is partition-aligned [128, F].
Each gate bank is ONE full-array matmul: the stationary is an 8-way
block-diagonal [128, 128] with per-chunk W.T blocks, a bias row on the
constant gap partition, and gap-column constants engineered so downstream
h-tile gap lanes hold an exact known constant (the next cell's bias 'one').

Precision: fp16 hi/lo 3-term split (W16*x16 + Wlo*x16 + W16*xlo) recovers
fp32-grade matmul accuracy at 16-bit PE speed. Sigmoid goes through tanh:
sigma(z) = (1+tanh(z/2))/2 with the 1/2 folded into weights, so only
Tanh+Exp ACT tables are used. selu is refactored to
(lambda/2 W)*(max(h2x,0) + 2a*exp(min(h2x,0)/2)) with the -a*lambda shift
folded into the m/d biases.
"""

from contextlib import ExitStack

import numpy as np

LAMBDA = 1.0507009873554805
ALPHA = 1.6732632423543772
AL = ALPHA * LAMBDA

B, IN, H = 1048576, 16, 12
NCORES = 8
R = B // NCORES          # rows per core
NCHUNK = 8               # chunk streams per core
CLEN = R // NCHUNK       # 16384 rows per stream
F = 512                  # free-dim tile
NIT = CLEN // F          # 32 iterations
GAP_A = 2.0              # gap bias for I and O banks
V2 = 1.25                # engineered H2 gap constant (fp16-exact)

_CACHED = {}


def _solve_gap_g(target):
    """Gap bias for the G bank so gap-lane h2x equals `target`."""
    t_a = np.tanh(GAP_A)
    tc = target / (1.0 + t_a)
    c2 = 2.0 * np.arctanh(tc)
    tg = c2 / (1.0 + t_a)
    assert abs(tg) < 1.0
    return float(np.arctanh(tg))


def _split16(a):
    hi = a.astype(np.float16)
    lo = (a - hi.astype(np.float64)).astype(np.float16)
    return hi, lo


def _stats_cell1(wb):
    """3 stationaries [128,128] fp16 for the x-input cell (no bias rows).
    Terms: W16*x_hi, Wlo*x_hi, W16*x_lo."""
    w16, wlo = _split16(wb)
    outs = []
    for w in (w16, wlo, w16):
        m = np.zeros((128, 128), np.float16)
        for j in range(NCHUNK):
            m[16 * j:16 * j + 16, 16 * j:16 * j + 12] = w.T
        outs.append(m)
    return outs


def _stats_hcell(wb, bb, gapval, vgap):
    """3 stationaries [128,128] fp16 for h-input cells.
    Terms: W16*H_hi (+bias rows), Wlo*H_hi (unused in 2-term mode),
    W16*H_lo. The bias rides H_hi gap lanes 12 AND 13 (both engineered to
    the constant `vgap`), split hi/lo across two stationary rows for
    fp32-grade bias accuracy. `gapval` seeds this cell's own gap columns."""
    w16, wlo = _split16(wb)
    bv = np.asarray(bb, np.float64) / vgap
    b16, blo = _split16(bv)
    outs = []
    for term, w in enumerate((w16, wlo, w16)):
        m = np.zeros((128, 128), np.float16)
        for j in range(NCHUNK):
            m[16 * j:16 * j + 12, 16 * j:16 * j + 12] = w.T
            if term == 0:
                m[16 * j + 12, 16 * j:16 * j + 12] = b16
                m[16 * j + 13, 16 * j:16 * j + 12] = blo
                m[16 * j + 12, 16 * j + 12:16 * j + 16] = gapval
        outs.append(m)
    return outs


def _prepare_consts(W_ih1, b_ih1, b_hh1, W_ih2, b_ih2, b_hh2,
                    W_ihm, b_ihm, b_hhm, W_ihd, b_ihd, b_hhd):
    i_s, g_s, o_s = slice(0, 12), slice(24, 36), slice(36, 48)
    g1gap = _solve_gap_g(1.0)    # H1 gap -> 1.0
    g2gap = _solve_gap_g(V2)     # H2c gap -> V2 (positive branch: h2x == H2c)

    b1 = (b_ih1 + b_hh1).astype(np.float64)
    b2 = (b_ih2 + b_hh2).astype(np.float64)
    # h2 = (lambda/2)*H2c + (al - al) ... with H2c centered the al-shift cancels:
    # W*h2 = (lambda/2)*W*H2c + al*sum(W) - al*sum(W) = (lambda/2)*W*H2c
    bm = (b_ihm + b_hhm).astype(np.float64)
    bd = (b_ihd + b_hhd).astype(np.float64)

    W1 = W_ih1.astype(np.float64)
    W2 = W_ih2.astype(np.float64)
    Wm = W_ihm.astype(np.float64)
    Wd = W_ihd.astype(np.float64)
    L2 = LAMBDA / 2.0

    # stat[(cell, bank, term)]; bank 0=I(sig), 1=G, 2=O(sig)
    stats = {}
    for bank, (gsl, sc) in enumerate(((i_s, 0.5), (g_s, 1.0), (o_s, 0.5))):
        gv1 = {0: GAP_A, 1: g1gap, 2: GAP_A}[bank]
        gv2 = {0: GAP_A, 1: g2gap, 2: GAP_A}[bank]
        stats[(0, bank)] = _stats_cell1(W1[gsl] * sc)
        # cell2 input H1 = 2*relu(h1) -> extra 1/2
        stats[(1, bank)] = _stats_hcell(W2[gsl] * (sc * 0.5), b2[gsl] * sc,
                                        gv2, 1.0)
        # m/d input H2'' -> scale lambda/2; gap lanes don't matter (0)
        stats[(2, bank)] = _stats_hcell(Wm[gsl] * (sc * L2), bm[gsl] * sc,
                                        0.0, V2)
        stats[(3, bank)] = _stats_hcell(Wd[gsl] * (sc * L2), bd[gsl] * sc,
                                        0.0, V2)

    w_np = np.zeros((128, 36 * 128), np.float16)
    for cell in range(4):
        for bank in range(3):
            for term in range(3):
                s = cell * 9 + bank * 3 + term
                w_np[:, 128 * s:128 * s + 128] = stats[(cell, bank)][term]

    # cell1 per-partition ACT bias: [128, 3] cols = I, G, O banks
    b_np = np.zeros((128, 3), np.float32)
    for j in range(NCHUNK):
        sl = slice(16 * j, 16 * j + 12)
        b_np[sl, 0] = b1[i_s] * 0.5
        b_np[sl, 1] = b1[g_s]
        b_np[sl, 2] = b1[o_s] * 0.5
        gp = slice(16 * j + 12, 16 * j + 16)
        b_np[gp, 0] = GAP_A
        b_np[gp, 1] = g1gap
        b_np[gp, 2] = GAP_A
    return w_np, b_np


def _build_bass():
    import concourse.bass as bass
    import concourse.mybir as mybir
    import concourse.tile as tile

    fp32 = mybir.dt.float32
    fp16 = mybir.dt.float16
    Tanh = mybir.ActivationFunctionType.Tanh
    Exp = mybir.ActivationFunctionType.Exp
    ADD = mybir.AluOpType.add
    MULT = mybir.AluOpType.mult
    MAX = mybir.AluOpType.max
    SUB = mybir.AluOpType.subtract
    TWOA = float(2.0 * ALPHA)

    nc = bass.Bass(name="dynet")
    xh_dev = nc.dram_tensor("xh_dev", [128, CLEN], fp16, kind="ExternalInput")
    w_dram = nc.dram_tensor("w_dram", [128, 36 * 128], fp16, kind="ExternalInput")
    b_dram = nc.dram_tensor("b_dram", [128, 3], fp32, kind="ExternalInput")
    m_dev = nc.dram_tensor("m_dev", [128, CLEN], fp16, kind="ExternalOutput")
    d_dev = nc.dram_tensor("d_dev", [128, CLEN], fp16, kind="ExternalOutput")

    with tile.TileContext(nc) as tc, ExitStack() as ctx:
        const_p = ctx.enter_context(tc.tile_pool(name="const", bufs=1))
        xp = ctx.enter_context(tc.tile_pool(name="x", bufs=3))
        Tp = ctx.enter_context(tc.tile_pool(name="T", bufs=3))
        smallp = ctx.enter_context(tc.tile_pool(name="small", bufs=4))
        mdp = ctx.enter_context(tc.tile_pool(name="md", bufs=3))
        hp = ctx.enter_context(tc.tile_pool(name="h", bufs=4))
        psp = ctx.enter_context(tc.tile_pool(name="ps", bufs=1, space="PSUM"))

        wsb = const_p.tile([128, 36 * 128], fp16)
        nc.sync.dma_start(wsb[:], w_dram[:])
        bsb = const_p.tile([128, 3], fp32)
        nc.sync.dma_start(bsb[:], b_dram[:])
        ebt = const_p.tile([128, 1], fp32)
        nc.vector.memset(ebt[:], float(np.log(2.0 * ALPHA)))

        # PSUM as a ring of 8 banks; each stage claims 3 consecutive (mod 8).
        # Rolling reuse keeps the PE ~2.7 stages ahead of the WAR horizon so
        # it never stalls long enough for the HAM clock gate to re-throttle.
        pp_all = psp.tile([128, 4096], fp32)

        def bank(s, b):
            w = ((3 * s + b) % 8) * 512
            return pp_all[:, w:w + 512]

        def mms(s, cell, rhs_hi):
            """3 full-array MMs: one fp16 term per gate bank (W16*hi with
            split bias rows; the lo terms are dropped — rel err stays well
            under the 2e-2 budget)."""
            for b in range(3):
                w = cell * 9 + b * 3
                nc.tensor.matmul(
                    bank(s, b), wsb[:, 128 * w:128 * w + 128], rhs_hi[:, :],
                    start=True, stop=True)

        def gate_act(s, T, bias_ap=None):
            """tanh over the stage's 3 ring banks -> T[:, 0:1536]."""
            base = (3 * s) % 8
            if bias_ap is not None:
                for b in range(3):
                    nc.scalar.activation(T[:, 512 * b:512 * b + 512],
                                         bank(s, b), Tanh,
                                         bias=bias_ap[:, b:b + 1])
                return
            n1 = min(3, 8 - base)
            nc.scalar.activation(
                T[:, 0:512 * n1],
                pp_all[:, 512 * base:512 * (base + n1)], Tanh)
            if n1 < 3:
                nc.scalar.activation(
                    T[:, 512 * n1:1536], pp_all[:, 0:512 * (3 - n1)], Tanh)

        xh = {}
        H1 = {}
        H2 = {}
        sctr = 0
        for k in range(NIT + 3):
            # prefetch x for iteration k
            if k < NIT:
                xh[k] = xp.tile([128, F], fp16, tag="xh", name=f"xh{k}")
                nc.sync.dma_start(xh[k][:], xh_dev[:, F * k:F * (k + 1)])

            # fused tanh input/output: [0:1024]=hmd(k-3), [1024:2048]=c2both(k)
            fin = smallp.tile([128, 2048], fp32, tag="fin", name=f"fin{k}")
            fout = smallp.tile([128, 2048], fp16, tag="fout", name=f"fout{k}")
            hmd = fin[:, 0:1024]
            omd = fout[:, 0:1024]
            c2both = fin[:, 1024:2048]
            tcboth = fout[:, 1024:2048]
            # --- stage m/d for iteration k-3 ---
            if 3 <= k:
                it = k - 3
                sm, sd = sctr, sctr + 1
                sctr += 2
                mms(sm, 2, H2[it])
                mms(sd, 3, H2[it])
                Tmd = Tp.tile([128, 3072], fp32, tag="Tmd", bufs=2)
                Tm = Tmd[:, 0:1536]
                Td = Tmd[:, 1536:3072]
                c2md = mdp.tile([128, 1024], fp32, tag="c2md")
                tcmd = mdp.tile([128, 1024], fp32, tag="tcmd")
                if (3 * sm) % 8 <= 2:
                    # m and d banks are 6 contiguous banks: one tanh op
                    base = (3 * sm) % 8
                    nc.scalar.activation(
                        Tmd[:, 0:3072],
                        pp_all[:, 512 * base:512 * base + 3072], Tanh)
                else:
                    gate_act(sm, Tm)
                    gate_act(sd, Td)
                nc.vector.scalar_tensor_tensor(
                    c2md[:, 0:512], Tm[:, 0:512], 1.0, Tm[:, 512:1024],
                    op0=ADD, op1=MULT)
                nc.vector.scalar_tensor_tensor(
                    c2md[:, 512:1024], Td[:, 0:512], 1.0, Td[:, 512:1024],
                    op0=ADD, op1=MULT)
                nc.scalar.activation(tcmd[:, :], c2md[:, :], Tanh, scale=0.5)
                nc.vector.scalar_tensor_tensor(
                    hmd[:, 0:512], Tm[:, 1024:1536], 1.0, tcmd[:, 0:512],
                    op0=ADD, op1=MULT)
                nc.vector.scalar_tensor_tensor(
                    hmd[:, 512:1024], Td[:, 1024:1536], 1.0, tcmd[:, 512:1024],
                    op0=ADD, op1=MULT)
                del H2[it]
                md_out_it = it


            # --- stage cell2 for iteration k-1 (produce c2, defer tc) ---
            if 1 <= k <= NIT:
                it = k - 1
                s2 = sctr
                sctr += 1
                mms(s2, 1, H1[it])
                T2 = Tp.tile([128, 1536], fp32, tag="T")
                gate_act(s2, T2)
                nc.vector.scalar_tensor_tensor(
                    c2both[:, 0:512], T2[:, 0:512], 1.0, T2[:, 512:1024],
                    op0=ADD, op1=MULT)

            # --- stage cell1 for iteration k ---
            if k < NIT:
                s1 = sctr
                sctr += 1
                mms(s1, 0, xh[k])
                T1 = Tp.tile([128, 1536], fp32, tag="T")
                gate_act(s1, T1, bias_ap=bsb)
                nc.vector.scalar_tensor_tensor(
                    c2both[:, 512:1024], T1[:, 0:512], 1.0, T1[:, 512:1024],
                    op0=ADD, op1=MULT)

            # merged tanh(0.5*x) over [hmd(k-3) | c2_2(k-1) | c2_1(k)]
            lo = 0 if 3 <= k else 1024
            hi = 2048 if k < NIT else (1536 if k <= NIT else 1024)
            if k == 0:
                lo = 1536
            nc.scalar.activation(fout[:, lo:hi], fin[:, lo:hi], Tanh,
                                 scale=0.5)
            if 3 <= k:
                it = md_out_it
                nc.sync.dma_start(m_dev[:, F * it:F * (it + 1)], omd[:, 0:512])
                nc.sync.dma_start(d_dev[:, F * it:F * (it + 1)], omd[:, 512:1024])

            # --- cell2 epilogue: h2x -> H2'' hi/lo ---
            if 1 <= k <= NIT:
                it = k - 1
                h2x2 = smallp.tile([128, F], fp32, tag="h2x2",
                                   name=f"h2x2_{k}")
                nc.vector.scalar_tensor_tensor(
                    h2x2[:, :], T2[:, 1024:1536], 1.0, tcboth[:, 0:512],
                    op0=ADD, op1=MULT)
                m0t = smallp.tile([128, F], fp32, tag="m0")
                e2t = smallp.tile([128, F], fp32, tag="e2")
                tmp2 = smallp.tile([128, F], fp32, tag="tmp2")
                h2h = hp.tile([128, F], fp16, tag="H2h", name=f"H2h_{it}")
                nc.vector.tensor_scalar_min(m0t[:, :], h2x2[:, :], 0.0)
                nc.scalar.activation(e2t[:, :], m0t[:, :], Exp,
                                     bias=ebt[:, :], scale=0.5)
                nc.vector.scalar_tensor_tensor(
                    tmp2[:, :], h2x2[:, :], 0.0, e2t[:, :], op0=MAX, op1=ADD)
                nc.vector.tensor_scalar_sub(h2h[:, :], tmp2[:, :], TWOA)
                H2[it] = h2h
                del H1[it]

            # --- cell1 epilogue: h2x -> H1 hi/lo ---
            if k < NIT:
                h2x1 = smallp.tile([128, F], fp32, tag="h2x1",
                                   name=f"h2x1_{k}")
                nc.vector.scalar_tensor_tensor(
                    h2x1[:, :], T1[:, 1024:1536], 1.0, tcboth[:, 512:1024],
                    op0=ADD, op1=MULT)
                h1h = hp.tile([128, F], fp16, tag="H1h", name=f"H1h_{k}")
                nc.vector.tensor_scalar_max(h1h[:, :], h2x1[:, :], 0.0)
                H1[k] = h1h
                del xh[k]

    _legalize_waits(nc)
    return nc


def _legalize_waits(nc):
    """Split multi-wait instructions into single-wait same-engine NoOps
    (the cayman ISA has one sync-wait slot per instruction)."""
    import concourse.mybir as mybir
    n = 0
    for func in nc.m.functions:
        for blk in func.blocks:
            out = []
            changed = False
            for inst in blk.instructions:
                si = inst.sync_info
                waits = list(si.on_wait) if si is not None and si.on_wait else []
                if len(waits) > 1:
                    changed = True
                    for w in waits[:-1]:
                        n += 1
                        nop = mybir.InstNoOp(name=f"legw-{n}", ins=[], outs=[])
                        nop.engine = inst.engine
                        nop.sync_info = mybir.SyncInfo(on_wait=[w], on_update=[])
                        out.append(nop)
                    inst.sync_info = mybir.SyncInfo(
                        on_wait=[waits[-1]],
                        on_update=list(si.on_update) if si.on_update else [])
                out.append(inst)
            if changed:
                blk.instructions = out
    return n


def _run(x, consts, trace=False):
    from concourse.bass_utils import run_bass_kernel_spmd

    if "nc" not in _CACHED:
        _CACHED["nc"] = _build_bass()
    nc = _CACHED["nc"]
    w_np, b_np = consts

    in_maps = []
    for c in range(NCORES):
        xs = x[c * R:(c + 1) * R].reshape(NCHUNK, CLEN, IN)
        x_devc = np.ascontiguousarray(
            xs.transpose(0, 2, 1)).reshape(128, CLEN)
        xhi = x_devc.astype(np.float16)
        in_maps.append({"xh_dev": xhi, "w_dram": w_np, "b_dram": b_np})

    res = run_bass_kernel_spmd(nc, in_maps, core_ids=list(range(NCORES)),
                               trace=trace)

    out = np.empty((2 * B, H), np.float32)
    for c in range(NCORES):
        for name, base in (("m_dev", 0), ("d_dev", B)):
            dev = res.results[c][name].astype(np.float32)  # [128, CLEN] fp16
            full = dev.reshape(NCHUNK, 16, CLEN)[:, :12, :]
            out[base + c * R: base + (c + 1) * R] = (
                full.transpose(0, 2, 1).reshape(R, H))
    return out, res


def kernel(x, W_ih1, b_ih1, b_hh1, W_ih2, b_ih2, b_hh2,
           W_ihm, b_ihm, b_hhm, W_ihd, b_ihd, b_hhd):
    x = np.asarray(x, np.float32)
    consts = _prepare_consts(
        np.asarray(W_ih1, np.float32), np.asarray(b_ih1, np.float32),
        np.asarray(b_hh1, np.float32), np.asarray(W_ih2, np.float32),
        np.asarray(b_ih2, np.float32), np.asarray(b_hh2, np.float32),
        np.asarray(W_ihm, np.float32), np.asarray(b_ihm, np.float32),
        np.asarray(b_hhm, np.float32), np.asarray(W_ihd, np.float32),
        np.asarray(b_ihd, np.float32), np.asarray(b_hhd, np.float32))
    out, _ = _run(x, consts, trace=False)
    return out



# revision 11
# speedup vs baseline: 1.4909x; 1.1184x over previous
"""DynamicsNet Trainium2 kernel: 4 zero-state LSTM cells, data-parallel on 8 cores.

Reference math per row x[16]:
    h1 = relu(lstm1(x));  h2 = selu(lstm2(h1));  m = tanh(lstmM(h2));
    d = tanh(lstmD(h2));  out = concat([m, d], axis=0)
(zero-state LSTM cell: h = sigmoid(o) * tanh(sigmoid(i) * tanh(g)), f unused)

Layout v2: per core, 10 chunk-streams at 12-lane stride (partitions 12j..12j+11)
plus ONE shared constant lane at partition 120 that carries bias into every
matmul (stationary row 120) and is gap-engineered to survive the activation
pipeline. vs the old 8x16 layout this processes 25% more rows per ACT column,
and the ACT engine is the bottleneck.

cell1's 16 input features don't fit 10 chunks in one 128-partition tile, so x
is split into xa (features 0-11 + const lane) and xb (features 12-15, 40
partitions) with the gate matmuls accumulating 2 terms.

Precision: single fp16 term everywhere (weights, x, h tiles) + fp16 outputs;
measured rel err ~1.5e-3 vs the 2e-2 budget. Sigmoid goes through tanh:
sigma(z) = (1+tanh(z/2))/2 with the 1/2 folded into weights, so only the
Tanh+Exp ACT table set is used. selu is refactored to
(lambda/2 W)*(max(h2x,0) + 2a*exp(min(h2x,0)/2) - 2a) with the -2a shift
folded into the m/d bias rows.
"""

from contextlib import ExitStack

import numpy as np

LAMBDA = 1.0507009873554805
ALPHA = 1.6732632423543772

B, IN, H = 1048576, 16, 12
NCORES = 8
R = B // NCORES          # real rows per core (131072)
NCHUNK = 10              # chunk streams per core
F = 512                  # free-dim tile
NIT = 26                 # iterations
CLEN = NIT * F           # 13312 rows per stream (10*13312 = 133120 >= R)
LS = 12                  # lane stride per chunk
PCONST = 120             # shared constant lane
GAP_A = 2.0              # gap bias for I and O banks
V2 = 1.25                # engineered H2 const-lane value (fp16-exact)

_CACHED = {}


def _solve_gap_g(target):
    """Gap bias for the G bank so the const lane's h2x equals `target`."""
    t_a = np.tanh(GAP_A)
    tc = target / (1.0 + t_a)
    c2 = 2.0 * np.arctanh(tc)
    tg = c2 / (1.0 + t_a)
    assert abs(tg) < 1.0
    return float(np.arctanh(tg))


def _prepare_consts(W_ih1, b_ih1, b_hh1, W_ih2, b_ih2, b_hh2,
                    W_ihm, b_ihm, b_hhm, W_ihd, b_ihd, b_hhd):
    i_s, g_s, o_s = slice(0, 12), slice(24, 36), slice(36, 48)
    g1gap = _solve_gap_g(1.0)    # H1 const lane -> 1.0
    g2gap = _solve_gap_g(V2)     # H2'' const lane -> V2 (positive branch)

    b1 = (b_ih1 + b_hh1).astype(np.float64)
    b2 = (b_ih2 + b_hh2).astype(np.float64)
    bm = (b_ihm + b_hhm).astype(np.float64)
    bd = (b_ihd + b_hhd).astype(np.float64)

    W1 = W_ih1.astype(np.float64)
    W2 = W_ih2.astype(np.float64)
    Wm = W_ihm.astype(np.float64)
    Wd = W_ihd.astype(np.float64)
    L2 = LAMBDA / 2.0

    # 15 stationary slots [128,128]: cell*3+bank for 4 cells, then 12+bank
    # for cell1's xb (features 12-15) term.
    w_np = np.zeros((128, 15 * 128), np.float16)

    def put(slot, m):
        w_np[:, 128 * slot:128 * slot + 128] = m.astype(np.float16)

    for bank, (gsl, sc) in enumerate(((i_s, 0.5), (g_s, 1.0), (o_s, 0.5))):
        gv1 = {0: GAP_A, 1: g1gap, 2: GAP_A}[bank]
        gv2 = {0: GAP_A, 1: g2gap, 2: GAP_A}[bank]

        # cell1 A-term: features 0-11, bias row, const-lane seed
        ma = np.zeros((128, 128), np.float64)
        mb = np.zeros((128, 128), np.float64)
        for j in range(NCHUNK):
            c = slice(LS * j, LS * j + 12)
            ma[LS * j:LS * j + 12, c] = (W1[gsl, 0:12] * sc).T
            mb[4 * j:4 * j + 4, c] = (W1[gsl, 12:16] * sc).T
            ma[PCONST, c] = b1[gsl] * sc
        ma[PCONST, PCONST] = gv1
        put(0 * 3 + bank, ma)
        put(12 + bank, mb)

        # cell2: input H1 = 2*relu(h1) -> extra 1/2; H1 const lane = 1.0
        m2 = np.zeros((128, 128), np.float64)
        for j in range(NCHUNK):
            c = slice(LS * j, LS * j + 12)
            m2[LS * j:LS * j + 12, c] = (W2[gsl] * (sc * 0.5)).T
            m2[PCONST, c] = b2[gsl] * sc
        m2[PCONST, PCONST] = gv2
        put(1 * 3 + bank, m2)

        # m/d: input H2'' -> scale lambda/2; H2'' const lane = V2
        for cell, (W, bb) in ((2, (Wm, bm)), (3, (Wd, bd))):
            mm = np.zeros((128, 128), np.float64)
            for j in range(NCHUNK):
                c = slice(LS * j, LS * j + 12)
                mm[LS * j:LS * j + 12, c] = (W[gsl] * (sc * L2)).T
                mm[PCONST, c] = bb[gsl] * sc / V2
            put(cell * 3 + bank, mm)
    return w_np


def _build_bass():
    import concourse.bass as bass
    import concourse.mybir as mybir
    import concourse.tile as tile

    fp32 = mybir.dt.float32
    fp16 = mybir.dt.float16
    Tanh = mybir.ActivationFunctionType.Tanh
    Exp = mybir.ActivationFunctionType.Exp
    ADD = mybir.AluOpType.add
    MULT = mybir.AluOpType.mult
    MAX = mybir.AluOpType.max
    TWOA = float(2.0 * ALPHA)

    nc = bass.Bass(name="dynet")
    xa_dev = nc.dram_tensor("xa_dev", [128, CLEN], fp16, kind="ExternalInput")
    xb_dev = nc.dram_tensor("xb_dev", [40, CLEN], fp16, kind="ExternalInput")
    w_dram = nc.dram_tensor("w_dram", [128, 15 * 128], fp16, kind="ExternalInput")
    m_dev = nc.dram_tensor("m_dev", [128, CLEN], fp16, kind="ExternalOutput")
    d_dev = nc.dram_tensor("d_dev", [128, CLEN], fp16, kind="ExternalOutput")

    with tile.TileContext(nc) as tc, ExitStack() as ctx:
        const_p = ctx.enter_context(tc.tile_pool(name="const", bufs=1))
        xp = ctx.enter_context(tc.tile_pool(name="x", bufs=3))
        Tp = ctx.enter_context(tc.tile_pool(name="T", bufs=3))
        smallp = ctx.enter_context(tc.tile_pool(name="small", bufs=4))
        hp = ctx.enter_context(tc.tile_pool(name="h", bufs=4))
        psp = ctx.enter_context(tc.tile_pool(name="ps", bufs=1, space="PSUM"))

        wsb = const_p.tile([128, 15 * 128], fp16)
        nc.sync.dma_start(wsb[:], w_dram[:])
        ebt = const_p.tile([128, 1], fp32)
        nc.vector.memset(ebt[:], float(np.log(2.0 * ALPHA)))

        # PSUM as a ring of 8 banks; each stage claims 3 consecutive (mod 8).
        pp_all = psp.tile([128, 4096], fp32)

        def bank(s, b):
            w = ((3 * s + b) % 8) * 512
            return pp_all[:, w:w + 512]

        def mms(s, cell, rhs):
            """3 full-array MMs: one fp16 term per gate bank."""
            for b in range(3):
                w = cell * 3 + b
                nc.tensor.matmul(
                    bank(s, b), wsb[:, 128 * w:128 * w + 128], rhs[:, :],
                    start=True, stop=True)

        def mms_c1(s, rhs_a, rhs_b):
            """cell1: 2 accumulating terms per bank (features 0-11 via xa
            incl bias/const rows, features 12-15 via xb on 40 partitions)."""
            for b in range(3):
                nc.tensor.matmul(
                    bank(s, b), wsb[:, 128 * b:128 * b + 128], rhs_a[:, :],
                    start=True, stop=False)
                nc.tensor.matmul(
                    bank(s, b), wsb[0:40, 128 * (12 + b):128 * (12 + b) + 128],
                    rhs_b[:, :], start=False, stop=True)

        def gate_act(s, T, n=3):
            """tanh over the stage's n ring banks -> T[:, 0:512*n]."""
            base = (3 * s) % 8
            n1 = min(n, 8 - base)
            nc.scalar.activation(
                T[:, 0:512 * n1],
                pp_all[:, 512 * base:512 * (base + n1)], Tanh)
            if n1 < n:
                nc.scalar.activation(
                    T[:, 512 * n1:512 * n], pp_all[:, 0:512 * (n - n1)], Tanh)

        xa = {}
        xb = {}
        H1 = {}
        H2 = {}
        TMD = {}
        sctr = 0
        # fin(k): [0:1024]=hmd(k-4), [1024:2048]=c2md(k-3),
        #         [2048:2560]=c2_2(k-1), [2560:3072]=c2_1(k)
        for k in range(NIT + 4):
            if k < NIT:
                xa[k] = xp.tile([128, F], fp16, tag="xa", name=f"xa{k}")
                nc.sync.dma_start(xa[k][:], xa_dev[:, F * k:F * (k + 1)])
                xb[k] = xp.tile([40, F], fp16, tag="xb", name=f"xb{k}")
                nc.sync.dma_start(xb[k][:], xb_dev[:, F * k:F * (k + 1)])

            fin = smallp.tile([128, 3072], fp32, tag="fin", name=f"fin{k}")
            fout = smallp.tile([128, 3072], fp16, tag="fout", name=f"fout{k}")

            # --- m/d matmuls + gates + c2 for iteration k-3 ---
            if 3 <= k < NIT + 3:
                it = k - 3
                sm, sd = sctr, sctr + 1
                sctr += 2
                mms(sm, 2, H2[it])
                mms(sd, 3, H2[it])
                Tmd = Tp.tile([128, 3072], fp32, tag="Tmd", bufs=3,
                              name=f"Tmd{it}")
                if (3 * sm) % 8 <= 2:
                    base = (3 * sm) % 8
                    nc.scalar.activation(
                        Tmd[:, 0:3072],
                        pp_all[:, 512 * base:512 * base + 3072], Tanh)
                else:
                    gate_act(sm, Tmd[:, 0:1536])
                    gate_act(sd, Tmd[:, 1536:3072])
                nc.vector.scalar_tensor_tensor(
                    fin[:, 1024:1536], Tmd[:, 0:512], 1.0, Tmd[:, 512:1024],
                    op0=ADD, op1=MULT)
                nc.vector.scalar_tensor_tensor(
                    fin[:, 1536:2048], Tmd[:, 1536:2048], 1.0,
                    Tmd[:, 2048:2560], op0=ADD, op1=MULT)
                TMD[it] = Tmd
                del H2[it]

            # --- hmd for iteration k-4 (needs tcmd from fout(k-1)) ---
            if 4 <= k:
                it2 = k - 4
                Tp_md = TMD.pop(it2)
                nc.vector.scalar_tensor_tensor(
                    fin[:, 0:512], Tp_md[:, 1024:1536], 1.0,
                    fprev[:, 1024:1536], op0=ADD, op1=MULT)
                nc.vector.scalar_tensor_tensor(
                    fin[:, 512:1024], Tp_md[:, 2560:3072], 1.0,
                    fprev[:, 1536:2048], op0=ADD, op1=MULT)

            # --- cell2 matmul + gates + c2 for iteration k-1 ---
            if 1 <= k <= NIT:
                it = k - 1
                s2 = sctr
                sctr += 1
                mms(s2, 1, H1[it])
                T2 = Tp.tile([128, 1536], fp32, tag="T")
                gate_act(s2, T2)
                nc.vector.scalar_tensor_tensor(
                    fin[:, 2048:2560], T2[:, 0:512], 1.0, T2[:, 512:1024],
                    op0=ADD, op1=MULT)

            # --- cell1 matmuls + gates + c2 for iteration k ---
            if k < NIT:
                s1 = sctr
                sctr += 1
                mms_c1(s1, xa[k], xb[k])
                T1 = Tp.tile([128, 1536], fp32, tag="T")
                gate_act(s1, T1)
                nc.vector.scalar_tensor_tensor(
                    fin[:, 2560:3072], T1[:, 0:512], 1.0, T1[:, 512:1024],
                    op0=ADD, op1=MULT)

            # --- ONE merged tanh(0.5*x) over all live fin ranges ---
            lo = 0 if 4 <= k else (1024 if k == 3 else
                                   (2048 if 1 <= k else 2560))
            hi = 3072 if k < NIT else (2560 if k == NIT else
                                       (2048 if k < NIT + 3 else 1024))
            nc.scalar.activation(fout[:, lo:hi], fin[:, lo:hi], Tanh,
                                 scale=0.5)
            if 4 <= k:
                it2 = k - 4
                nc.sync.dma_start(m_dev[:, F * it2:F * (it2 + 1)],
                                  fout[:, 0:512])
                nc.sync.dma_start(d_dev[:, F * it2:F * (it2 + 1)],
                                  fout[:, 512:1024])

            # --- cell2 epilogue: h2x -> H2'' (selu refactor) ---
            if 1 <= k <= NIT:
                it = k - 1
                h2x2 = smallp.tile([128, F], fp32, tag="h2x2",
                                   name=f"h2x2_{k}")
                nc.vector.scalar_tensor_tensor(
                    h2x2[:, :], T2[:, 1024:1536], 1.0, fout[:, 2048:2560],
                    op0=ADD, op1=MULT)
                m0t = smallp.tile([128, F], fp32, tag="m0")
                e2t = smallp.tile([128, F], fp32, tag="e2")
                tmp2 = smallp.tile([128, F], fp32, tag="tmp2")
                h2h = hp.tile([128, F], fp16, tag="H2h", name=f"H2h_{it}")
                nc.vector.tensor_scalar_min(m0t[:, :], h2x2[:, :], 0.0)
                nc.scalar.activation(e2t[:, :], m0t[:, :], Exp,
                                     bias=ebt[:, :], scale=0.5)
                nc.vector.scalar_tensor_tensor(
                    tmp2[:, :], h2x2[:, :], 0.0, e2t[:, :], op0=MAX, op1=ADD)
                nc.vector.tensor_scalar_sub(h2h[:, :], tmp2[:, :], TWOA)
                H2[it] = h2h
                del H1[it]

            # --- cell1 epilogue: h2x -> H1 ---
            if k < NIT:
                h2x1 = smallp.tile([128, F], fp32, tag="h2x1",
                                   name=f"h2x1_{k}")
                nc.vector.scalar_tensor_tensor(
                    h2x1[:, :], T1[:, 1024:1536], 1.0, fout[:, 2560:3072],
                    op0=ADD, op1=MULT)
                h1h = hp.tile([128, F], fp16, tag="H1h", name=f"H1h_{k}")
                nc.vector.tensor_scalar_max(h1h[:, :], h2x1[:, :], 0.0)
                H1[k] = h1h
                del xa[k], xb[k]

            fprev = fout

    _legalize_waits(nc)
    return nc


def _legalize_waits(nc):
    """Split multi-wait instructions into single-wait same-engine NoOps
    (the cayman ISA has one sync-wait slot per instruction)."""
    import concourse.mybir as mybir
    n = 0
    for func in nc.m.functions:
        for blk in func.blocks:
            out = []
            changed = False
            for inst in blk.instructions:
                si = inst.sync_info
                waits = list(si.on_wait) if si is not None and si.on_wait else []
                if len(waits) > 1:
                    changed = True
                    for w in waits[:-1]:
                        n += 1
                        nop = mybir.InstNoOp(name=f"legw-{n}", ins=[], outs=[])
                        nop.engine = inst.engine
                        nop.sync_info = mybir.SyncInfo(on_wait=[w], on_update=[])
                        out.append(nop)
                    inst.sync_info = mybir.SyncInfo(
                        on_wait=[waits[-1]],
                        on_update=list(si.on_update) if si.on_update else [])
                out.append(inst)
            if changed:
                blk.instructions = out
    return n


def _run(x, consts, trace=False):
    from concourse.bass_utils import run_bass_kernel_spmd

    if "nc" not in _CACHED:
        _CACHED["nc"] = _build_bass()
    nc = _CACHED["nc"]
    w_np = consts

    in_maps = []
    for c in range(NCORES):
        xpad = np.zeros((NCHUNK * CLEN, IN), np.float32)
        xpad[:R] = x[c * R:(c + 1) * R]
        arr = np.ascontiguousarray(
            xpad.reshape(NCHUNK, CLEN, IN).transpose(0, 2, 1))  # [C,16,CLEN]
        xa = np.zeros((128, CLEN), np.float16)
        xb = np.zeros((40, CLEN), np.float16)
        for j in range(NCHUNK):
            xa[LS * j:LS * j + 12] = arr[j, 0:12]
            xb[4 * j:4 * j + 4] = arr[j, 12:16]
        xa[PCONST] = 1.0
        in_maps.append({"xa_dev": xa, "xb_dev": xb, "w_dram": w_np})

    res = run_bass_kernel_spmd(nc, in_maps, core_ids=list(range(NCORES)),
                               trace=trace)

    out = np.empty((2 * B, H), np.float32)
    lanes = np.concatenate([np.arange(LS * j, LS * j + 12)
                            for j in range(NCHUNK)])
    for c in range(NCORES):
        for name, base in (("m_dev", 0), ("d_dev", B)):
            dev = res.results[c][name].astype(np.float32)  # [128, CLEN]
            full = dev[lanes].reshape(NCHUNK, 12, CLEN)
            rows = full.transpose(0, 2, 1).reshape(NCHUNK * CLEN, H)
            out[base + c * R: base + (c + 1) * R] = rows[:R]
    return out, res


def kernel(x, W_ih1, b_ih1, b_hh1, W_ih2, b_ih2, b_hh2,
           W_ihm, b_ihm, b_hhm, W_ihd, b_ihd, b_hhd):
    x = np.asarray(x, np.float32)
    consts = _prepare_consts(
        np.asarray(W_ih1, np.float32), np.asarray(b_ih1, np.float32),
        np.asarray(b_hh1, np.float32), np.asarray(W_ih2, np.float32),
        np.asarray(b_ih2, np.float32), np.asarray(b_hh2, np.float32),
        np.asarray(W_ihm, np.float32), np.asarray(b_ihm, np.float32),
        np.asarray(b_hhm, np.float32), np.asarray(W_ihd, np.float32),
        np.asarray(b_ihd, np.float32), np.asarray(b_hhd, np.float32))
    out, _ = _run(x, consts, trace=False)
    return out
